# revision 29
# baseline (speedup 1.0000x reference)
"""AreaAttention Trainium2 kernel (v2).

Data-parallel over batch: 8 batches -> 8 NeuronCores, one batch per core.

Reference semantics recap.  The torch-style `qk.reshape(B*AREA, 2C, N//AREA)`
is row-major, so area a draws from original channels [128a, 128a+128) (q from
the first 64, k from the next 64) and attention feature d within a head mixes
channel offset j in [0,8) with spatial quarter g = n//1024 as d = 4j + g;
tokens are n' = n % 1024.  For v (256 channels): channel 64a + 8h + j, same
(g, n') split, d = 4j + g.  The attention output for (a, h, d, n') lands at
channel 64a + 8h + j, spatial n = 1024g + n'.

Per core pipeline (engines in parentheses):
  1. qk conv in fp8 DoubleRow (PE, 1 matmul per 512-chunk), DVE affine
     epilogue to f16 `stage`; ONE mismatched-shape SBUF->SBUF DMA per
     (area, head) regroups stage rows [8, 4096] into qkr [32 = (j,g), 1024].
  2. vT conv in fp8 DoubleRow, transposed orientation (x8^T @ w_v8^T); DVE
     epilogue scales by s_v/16 and writes fp8 directly into the AV lhsT
     layout vTr[m', j-pair, j-parity, a, h, d] (+ ones column for fused Z).
  3. v4 conv + depthwise 3x3 via diagonal-matrix matmuls in f16 (exact pe
     path; b_v + b_pe folded in the epilogue bias).
  4. attention per (a, head-pair): K^T Q in f16 (row-packed K=32, cost-free
     packing via tile_position) -> exp on ACT with scale=1/sqrt(32),
     bias=-ESHIFT, writing fp8e4m3 (max score*SCALE is 7.74, so e^(s-3)
     tops out ~115 << 448) -> O/Z via fp8 DoubleRow matmuls [V;1]^T @ expT.
     Epilogue: 1/Z with one strided reciprocal_approx_fast from PSUM,
     stream_shuffle broadcast, two PSUM->f16 normalize multiplies, and ONE
     scatter DMA per head back to (channel, spatial) in t.
  5. t += pe per area, then proj conv (f16) + affine into the output.

The shift ESHIFT cancels in O/Z (softmax shift invariance); it only guards
fp8 overflow.  b_v is omitted from vTr: softmax(w)@(v+b_v) = softmax(w)@v
+ b_v, and b_v is added back via the pe-path bias (b_pe + b_v).
"""

import sys

try:
    import concourse  # noqa: F401
except ImportError:  # pragma: no cover
    sys.path.insert(0, "/opt/trn_rl_repo")

from contextlib import ExitStack

import numpy as np

import concourse.bass as bass
import concourse.mybir as mybir
import concourse.tile as tile
from concourse import bacc

P = 128
C = 256
H = W = 64
N = H * W          # 4096
AREA = 4
NA = N // AREA     # 1024
HEADS = 8
HD = 32
SCALE = float(HD) ** -0.5
ESHIFT = 3.0       # exp(s*SCALE - ESHIFT); max s*SCALE ~ 7.74, fp8e4 max 448
B = 8
W8SCALE = 16.0     # fp8 weights stored *16, epilogue scales fold /16

f32 = mybir.dt.float32
f16 = mybir.dt.float16
f8 = mybir.dt.float8e4
i32 = mybir.dt.int32

Exp = mybir.ActivationFunctionType.Exp
Ident = mybir.ActivationFunctionType.Identity
MULT = mybir.AluOpType.mult
ADD = mybir.AluOpType.add
DR = mybir.MatmulPerfMode.DoubleRow

# Schraudolph fast-exp constants (j=7 exp tiles run on DVE to offload the
# ACT bottleneck): i32 = round(A*s + B); bitcast is ~exp(s*SCALE - ESHIFT)
# with ~1.5% rms / 4.4% max rel err -- same order as the fp8e4m3
# quantization of the output, and softmax-averaged in the output.
_L2E23 = float(2 ** 23) / float(np.log(2.0))
A_SCH = _L2E23 * SCALE
B_SCH = float(127 * 2 ** 23 - 545947) - _L2E23 * ESHIFT


def build_module(repeat=1):
    """Build the kernel module.  repeat>1 wraps the whole body in a
    hardware For_i loop executing the identical computation `repeat`
    times (used by test.py to measure per-execution HW time from single
    dispatches; the graded module uses repeat=1 with identical body)."""
    import contextlib

    nc = bacc.Bacc("TRN2", target_bir_lowering=False, debug=False)

    # ---- DRAM I/O ------------------------------------------------------
    xb = nc.dram_tensor("xb", [P, 2, N], f16, kind="ExternalInput").ap()
    xb8 = nc.dram_tensor("xb8", [P, 2, N], f8, kind="ExternalInput").ap()
    w8 = nc.dram_tensor("w8", [P, 1536], f8, kind="ExternalInput").ap()
    wf16 = nc.dram_tensor("wf16", [P, 3328], f16, kind="ExternalInput").ap()
    cf32 = nc.dram_tensor("cf32", [P, 280], f32, kind="ExternalInput").ap()
    out = nc.dram_tensor("out", [P, 2, N], f32, kind="ExternalOutput").ap()

    with tile.TileContext(nc) as tc, ExitStack() as ctx:
        const = ctx.enter_context(tc.tile_pool(name="const", bufs=1))
        big = ctx.enter_context(tc.tile_pool(name="big", bufs=1))
        work = ctx.enter_context(tc.tile_pool(name="work", bufs=1))
        ph1 = ctx.enter_context(tc.tile_pool(name="ph1", bufs=1))
        psc = ctx.enter_context(tc.tile_pool(name="psc", bufs=2, space="PSUM"))
        pst = ctx.enter_context(tc.tile_pool(name="pst", bufs=2, space="PSUM"))
        psa = ctx.enter_context(tc.tile_pool(name="psa", bufs=1, space="PSUM"))

        rep_cm = tc.For_i(0, repeat) if repeat > 1 else contextlib.nullcontext()
        with rep_cm:
            # ---- constants / weights in SBUF ----------------------------
            w8_sb = const.tile([P, 1536], f8)
            nc.sync.dma_start(w8_sb[:], w8)
            cf32_sb = const.tile([P, 280], f32)
            nc.sync.dma_start(cf32_sb[:], cf32)

            wqk8_sb = w8_sb[:, 0:1024].rearrange("p (k m) -> p k m", k=2)
            wv8_sb = w8_sb[:, 1024:1536].rearrange("p (k m) -> p k m", k=2)

            svb_sb = cf32_sb[:, 0:256]
            sqk_sb = cf32_sb[:, 256:260]
            bqk_sb = cf32_sb[:, 260:264]
            sv_sb = cf32_sb[:, 264:266]
            bv_sb = cf32_sb[:, 266:268]
            spe_sb = cf32_sb[:, 268:270]
            bpe2_sb = cf32_sb[:, 270:272]
            spj_sb = cf32_sb[:, 272:274]
            bpj_sb = cf32_sb[:, 274:276]
            esh_sb = cf32_sb[:, 276:277]   # holds -ESHIFT

            # ---- persistent activations ---------------------------------
            # qkr: [p = 32*(h%4) + d, half(q/k), area, h//4, n'], d = 4j+g
            qkr = big.tile([P, 2, AREA, 2, NA], f16)
            # vTr: [p = m'%128, j-pair, j-parity, area, head, d].  d-extent
            # is 64 (DoubleRow dst partition count must be 16/32/64/128 at
            # base 0): cols 0:32 = V, col 32 = ones (fused Z), 33:64 zeros.
            vTr = big.tile([P, 4, 2, AREA, HEADS, 64], f8)
            nc.gpsimd.memset(vTr[:, :, :, :, :, 32:64], 0.0)
            nc.gpsimd.memset(vTr[:, :, :, :, :, 32:33], 1.0)
            pe_sb = big.tile([P, 2, N], f16)
            t_sb = big.tile([P, 2, N], f16)
            projpe = big.tile([P, 2, N], f16)

            xb8_sb = ph1.tile([P, 2, N], f8)
            nc.sync.dma_start(xb8_sb[:], xb8)
            wf16_sb = const.tile([P, 3328], f16)
            nc.sync.dma_start(wf16_sb[:], wf16)
            xb_sb = ph1.tile([P, 2, N], f16)
            nc.sync.dma_start(xb_sb[:], xb)

            wvT_sb = wf16_sb[:, 0:512].rearrange("p (k m) -> p k m", k=2)
            wpjT_sb = wf16_sb[:, 512:1024].rearrange("p (k m) -> p k m", k=2)
            dw_sb = wf16_sb[:, 1024:3328].rearrange(
                "p (c t m) -> p c t m", c=2, t=9
            )

            v4pad = ph1.tile([P, 2, H + 2, W + 2], f16)

            svb_r = svb_sb.rearrange("p (a h j) -> p a h j", a=AREA, h=HEADS)

            def qk_conv(tau, act_epi=False):
                # fp8 DoubleRow conv for permuted-channel tile tau
                # (tau = 2*half + a//2; partition = 64*(a%2) + 8h + j).
                # act_epi alternates epilogue chunks onto the (startup-idle)
                # ACT engine to halve the serial epilogue latency before the
                # first exp can issue.
                stage = ph1.tile([P, N], f16, tag="stage", bufs=2,
                                 name=f"st{tau}")
                for nk in range(8):
                    cps = psc.tile([P, 512], f32, tag="cps",
                                   name=f"qk{tau}_{nk}")
                    nc.tensor.matmul(
                        cps[:],
                        wqk8_sb[:, :, 128 * tau : 128 * (tau + 1)],
                        xb8_sb[:, :, 512 * nk : 512 * (nk + 1)],
                        start=True,
                        stop=True,
                        perf_mode=DR,
                    )
                    if act_epi and nk % 2 == 1:
                        nc.scalar.activation(
                            stage[:, 512 * nk : 512 * (nk + 1)],
                            cps[:],
                            Ident,
                            bias=bqk_sb[:, tau : tau + 1],
                            scale=sqk_sb[:, tau : tau + 1],
                        )
                    else:
                        nc.vector.tensor_scalar(
                            stage[:, 512 * nk : 512 * (nk + 1)],
                            cps[:],
                            sqk_sb[:, tau : tau + 1],
                            bqk_sb[:, tau : tau + 1],
                            MULT,
                            ADD,
                        )
                return stage

            def qk_conv_pair(tq, tk, act_epi=False):
                # conv both halves, then regroup DMAs interleaved by (a, h)
                # so the first pair's q AND k tiles land as early as possible.
                stq = qk_conv(tq, act_epi)
                stk = qk_conv(tk, act_epi)
                ah = tq % 2
                for a in (2 * ah, 2 * ah + 1):
                    for h in range(HEADS):
                        for tau, st in ((tq, stq), (tk, stk)):
                            nc.sync.dma_start(
                                qkr[
                                    32 * (h % 4) : 32 * (h % 4) + 32,
                                    tau // 2, a, h // 4, :,
                                ],
                                st[
                                    64 * (a % 2) + 8 * h : 64 * (a % 2) + 8 * h + 8,
                                    :,
                                ],
                            )

            def vt_conv():
                # fp8 DoubleRow vT conv; spatial n = 128t + p with t = 8g+jj.
                for t in range(N // P):
                    g, jj = t // 8, t % 8
                    cps = psc.tile([P, 512], f32, tag="cps", name=f"vt{t}")
                    vtp = cps[:, 0:256]
                    nc.tensor.matmul(
                        vtp,
                        xb8_sb[:, :, 128 * t : 128 * (t + 1)],
                        wv8_sb[:],
                        start=True,
                        stop=True,
                        perf_mode=DR,
                    )
                    nc.vector.tensor_tensor(
                        vTr[:, jj // 2, jj % 2, :, :, g : g + 29 : 4],
                        vtp.rearrange("p (a h j) -> p a h j", a=AREA, h=HEADS),
                        svb_r,
                        MULT,
                    )

            def v4_conv():
                # border-only zeroing of the padded image, then v4 conv
                # (f16, normal orientation); DVE epilogues.
                nc.vector.memset(v4pad[:, :, 0, :], 0.0)
                nc.vector.memset(v4pad[:, :, H + 1, :], 0.0)
                nc.vector.memset(v4pad[:, :, 1 : H + 1, 0:1], 0.0)
                nc.vector.memset(v4pad[:, :, 1 : H + 1, W + 1 : W + 2], 0.0)
                for ct in range(2):
                    for rc in range(8):
                        cps = psc.tile([P, 512], f32, tag="cps",
                                       name=f"v4_{ct}_{rc}")
                        for kt in range(2):
                            nc.tensor.matmul(
                                cps[:],
                                wvT_sb[:, kt, 128 * ct : 128 * (ct + 1)],
                                xb_sb[:, kt, 512 * rc : 512 * (rc + 1)],
                                start=(kt == 0),
                                stop=(kt == 1),
                            )
                        nc.vector.tensor_scalar(
                            v4pad[:, ct, 1 + 8 * rc : 1 + 8 * (rc + 1), 1 : 1 + W],
                            cps[:],
                            sv_sb[:, ct : ct + 1],
                            bv_sb[:, ct : ct + 1],
                            MULT,
                            ADD,
                        )

            def dw_conv(ct):
                # depthwise 3x3 via diagonal matmuls; DVE epilogues.
                for rc in range(8):
                    cps = psc.tile([P, 512], f32, tag="cps",
                                   name=f"dw_{ct}_{rc}")
                    for tap in range(9):
                        dy, dx = tap // 3 - 1, tap % 3 - 1
                        rhs = v4pad[
                            :,
                            ct,
                            1 + dy + 8 * rc : 1 + dy + 8 * (rc + 1),
                            1 + dx : 1 + dx + W,
                        ]
                        nc.tensor.matmul(
                            cps[:],
                            dw_sb[:, ct, tap, :],
                            rhs,
                            start=(tap == 0),
                            stop=(tap == 8),
                        )
                    nc.vector.tensor_scalar(
                        pe_sb[:, ct, 512 * rc : 512 * (rc + 1)],
                        cps[:],
                        spe_sb[:, ct : ct + 1],
                        bpe2_sb[:, ct : ct + 1],
                        MULT,
                        ADD,
                    )

            def proj_pe(ct):
                # proj(t + pe) = proj(t) + proj(pe): the pe half runs early
                # (during areas 2/3) into an SBUF f32 staging buffer with
                # s_proj/b_proj folded; the t half accumulates on top in the
                # tail epilogue.
                for nq in range(8):
                    pj = psc.tile([P, 512], f32, tag="cps",
                                  name=f"pjpe_{ct}_{nq}")
                    for kt in range(2):
                        nc.tensor.matmul(
                            pj[:],
                            wpjT_sb[:, kt, 128 * ct : 128 * (ct + 1)],
                            pe_sb[:, kt, 512 * nq : 512 * (nq + 1)],
                            start=(kt == 0),
                            stop=(kt == 1),
                        )
                    nc.vector.tensor_scalar(
                        projpe[:, ct, 512 * nq : 512 * (nq + 1)],
                        pj[:],
                        spj_sb[:, ct : ct + 1],
                        bpj_sb[:, ct : ct + 1],
                        MULT,
                        ADD,
                    )

            def pair_epilogue(a, pr, nh, avA, avB):
                # per-nh half: rZ, broadcast, normalize, scatter.  Halving
                # shortens the end-of-kernel critical chain (the nh=0 half
                # overlaps the nh=1 exp stream).
                sl = slice(512 * nh, 512 * (nh + 1))
                # single-partition reciprocal_approx_fast returns garbage on
                # HW, so stage both Z rows into one tile (rows 0/32) and run
                # one multi-row op over [0:33] (rows 1..31 are don't-care).
                rzt = work.tile([64, 512], f32, tag="rzt", bufs=1,
                                name=f"rzt{a}{pr}{nh}")
                nc.vector.tensor_copy(rzt[0:1, :], avA[32:33, :])
                nc.vector.tensor_copy(rzt[32:33, :], avB[32:33, :])
                rzr = work.tile([64, 512], f32, tag="rzr", bufs=1,
                                name=f"rzr{a}{pr}{nh}")
                nc.vector.reciprocal_approx_fast(rzr[0:33, :], rzt[0:33, :])
                rzb = work.tile([64, 512], f32, tag="rzb", bufs=1,
                                name=f"rzb{a}{pr}{nh}")
                nc.vector.stream_shuffle(rzb[:], rzr[:], [0] * 32)
                o16 = work.tile([64, 512], f16, tag="o16", bufs=2,
                                name=f"o16{a}{pr}{nh}")
                nc.vector.tensor_tensor(
                    o16[0:32, :], avA[0:32, :], rzb[0:32, :], MULT
                )
                nc.vector.tensor_tensor(
                    o16[32:64, :], avB[0:32, :], rzb[32:64, :], MULT
                )
                # one scatter DMA per head: o16[4j+g, n'] ->
                # t[64*(a%2)+8h+j, a//2, 1024g+n'] (flat-order pairing).
                for q, h in ((0, 2 * pr), (1, 2 * pr + 1)):
                    nc.sync.dma_start(
                        t_sb[
                            64 * (a % 2) + 8 * h : 64 * (a % 2) + 8 * h + 8,
                            a // 2, :,
                        ].rearrange("p (g n) -> p g n", g=4)[:, :, sl],
                        o16[32 * q : 32 * q + 32, :],
                    )

            def attention_area(a):
                for pr in range(4):
                    hA, hB = 2 * pr, 2 * pr + 1
                    # emit BOTH nh-halves' QK+exp before any AV matmul: the
                    # PE queue is in-order, so AVs ahead of the next half's
                    # QK would stall the ACT exp stream at each half
                    # boundary.
                    e8all = []
                    for nh in range(2):
                        for j in range(8):
                            if j % 2 == 0:
                                e8 = work.tile([P, 2, NA], f8, tag="e8",
                                               bufs=10,
                                               name=f"e{a}{pr}{nh}{j // 2}")
                                e8all.append(e8)
                            Tp = pst.tile([P, NA], f32, tag="Tp",
                                          name=f"Tp{a}{pr}{nh}{j}")
                            for h, cb in ((hA, 0), (hB, 512)):
                                rb = 32 * (h % 4)
                                nc.tensor.matmul(
                                    Tp[:, cb : cb + 512],
                                    qkr[rb : rb + 32, 1, a, h // 4,
                                        128 * j : 128 * (j + 1)],
                                    qkr[rb : rb + 32, 0, a, h // 4,
                                        512 * nh : 512 * (nh + 1)],
                                    start=True,
                                    stop=True,
                                    tile_position=(rb, 0),
                                )
                            offload = (
                                (j == 7 and not (pr == 0 and nh == 0))
                                or (j == 3 and a % 2 == 0 and nh == 1)
                            )
                            if offload:
                                # offload some exps from the ACT bottleneck
                                # to DVE via the Schraudolph bit trick
                                # (verified on HW); the SBUF->SBUF fp8
                                # convert rides the otherwise-idle Pool
                                # engine.
                                yi = work.tile([P, NA], i32, tag="yi",
                                               bufs=2,
                                               name=f"yi{a}{pr}{nh}{j}")
                                nc.vector.tensor_scalar(
                                    yi[:], Tp[:], A_SCH, B_SCH, MULT, ADD
                                )
                                nc.gpsimd.tensor_copy(
                                    e8[:, j % 2, :], yi[:].bitcast(f32)
                                )
                            else:
                                nc.scalar.activation(
                                    e8[:, j % 2, :], Tp[:], Exp,
                                    bias=esh_sb, scale=SCALE,
                                )
                    for nh in range(2):
                        avA = psa.tile([64, 512], f32, tag="av", bufs=2,
                                       name=f"avA{a}_{pr}_{nh}")
                        avB = psa.tile([64, 512], f32, tag="av", bufs=2,
                                       name=f"avB{a}_{pr}_{nh}")
                        for jp in range(4):
                            for h, cb, av_ in ((hA, 0, avA), (hB, 512, avB)):
                                nc.tensor.matmul(
                                    av_[:, :],
                                    vTr[:, jp, :, a, h, :],
                                    e8all[4 * nh + jp][:, :, cb : cb + 512],
                                    start=(jp == 0),
                                    stop=(jp == 3),
                                    perf_mode=DR,
                                    skip_group_check=True,
                                )
                        pair_epilogue(a, pr, nh, avA, avB)

            # areas 0/1 need qk taus 0 (q) and 2 (k); areas 2/3 need 1 and 3.
            # pe-path work (v4, dw, proj_pe) is emitted between attention
            # areas so its PE time drains under the ACT-bound exp stream.
            qk_conv_pair(0, 2, act_epi=True)
            vt_conv()
            attention_area(0)
            qk_conv_pair(1, 3)
            attention_area(1)
            v4_conv()
            dw_conv(0)
            attention_area(2)
            dw_conv(1)
            proj_pe(0)
            proj_pe(1)
            attention_area(3)

            # ---- proj conv, t half (needs all areas' t) -----------------
            for ct in range(2):
                for nq in range(8):
                    pj = psc.tile([P, 512], f32, tag="cps",
                                  name=f"pj_{ct}_{nq}")
                    for kt in range(2):
                        nc.tensor.matmul(
                            pj[:],
                            wpjT_sb[:, kt, 128 * ct : 128 * (ct + 1)],
                            t_sb[:, kt, 512 * nq : 512 * (nq + 1)],
                            start=(kt == 0),
                            stop=(kt == 1),
                        )
                    outc = work.tile([P, 512], f32, tag="outc", bufs=4)
                    nc.vector.scalar_tensor_tensor(
                        outc[:],
                        pj[:],
                        spj_sb[:, ct : ct + 1],
                        projpe[:, ct, 512 * nq : 512 * (nq + 1)],
                        MULT,
                        ADD,
                    )
                    nc.sync.dma_start(
                        out[:, ct, 512 * nq : 512 * (nq + 1)], outc[:]
                    )
    nc.compile()
    return nc


def make_in_maps(x, w_qk, s_qk, b_qk, w_v, s_v, b_v, w_pe, s_pe, b_pe,
                 w_proj, s_proj, b_proj):
    """Host-side sharding + layout prep. Returns list of 8 per-core dicts."""
    f8np = mybir.dt.np(f8)
    x = np.asarray(x, dtype=np.float32)
    w_qk = np.asarray(w_qk, dtype=np.float32)
    w_v = np.asarray(w_v, dtype=np.float32)
    w_pe = np.asarray(w_pe, dtype=np.float32)
    w_proj = np.asarray(w_proj, dtype=np.float32)
    s_qk, b_qk = np.asarray(s_qk, np.float32), np.asarray(b_qk, np.float32)
    s_v, b_v = np.asarray(s_v, np.float32), np.asarray(b_v, np.float32)
    s_pe, b_pe = np.asarray(s_pe, np.float32), np.asarray(b_pe, np.float32)
    s_proj, b_proj = np.asarray(s_proj, np.float32), np.asarray(b_proj, np.float32)

    # permute qk conv outputs: new channel (half, a, h, j) <- 128a+64*half+8h+j
    perm = np.empty(2 * C, np.int64)
    i = 0
    for half in range(2):
        for a in range(AREA):
            for h in range(HEADS):
                for j in range(8):
                    perm[i] = 128 * a + 64 * half + 8 * h + j
                    i += 1
    w_qk_p, s_qk_p, b_qk_p = w_qk[perm], s_qk[perm], b_qk[perm]

    def ptile2(w):  # [256, M] -> [128, 2, M]
        k, m = w.shape
        return np.ascontiguousarray(w.reshape(2, P, m).transpose(1, 0, 2))

    wpe9 = w_pe.reshape(C, 9).astype(np.float16)
    dwd = np.zeros((P, 2, 9, P), np.float16)
    for ct in range(2):
        for tap in range(9):
            np.fill_diagonal(dwd[:, ct, tap, :], wpe9[128 * ct : 128 * (ct + 1), tap])

    w8 = np.concatenate(
        [
            ptile2(w_qk_p.T * W8SCALE).astype(f8np).reshape(P, -1),
            ptile2(w_v.T * W8SCALE).astype(f8np).reshape(P, -1),
        ],
        axis=1,
    )
    wf16 = np.concatenate(
        [
            ptile2(w_v.T).astype(np.float16).reshape(P, -1),
            ptile2(w_proj.T).astype(np.float16).reshape(P, -1),
            dwd.reshape(P, -1),
        ],
        axis=1,
    )
    cf32 = np.concatenate(
        [
            np.broadcast_to((s_v / W8SCALE).reshape(1, C), (P, C)),
            (s_qk_p / W8SCALE).reshape(4, P).T,
            b_qk_p.reshape(4, P).T,
            s_v.reshape(2, P).T,
            b_v.reshape(2, P).T,
            s_pe.reshape(2, P).T,
            (b_pe + b_v).reshape(2, P).T,
            s_proj.reshape(2, P).T,
            b_proj.reshape(2, P).T,
            np.full((P, 1), -ESHIFT, np.float32),
            np.zeros((P, 3), np.float32),
        ],
        axis=1,
    ).astype(np.float32)
    shared = {
        "w8": np.ascontiguousarray(w8),
        "wf16": np.ascontiguousarray(wf16),
        "cf32": np.ascontiguousarray(cf32),
    }
    in_maps = []
    for b in range(B):
        m = dict(shared)
        xr = np.ascontiguousarray(x[b].reshape(2, P, N).transpose(1, 0, 2))
        m["xb"] = xr.astype(np.float16)
        m["xb8"] = xr.astype(f8np)
        in_maps.append(m)
    return in_maps


def assemble_output(results):
    """results: list of 8 per-core dicts with 'out' [128, 2, N] fp32."""
    outs = []
    for b in range(B):
        ob = results[b]["out"]  # [128, 2, N]
        outs.append(ob.transpose(1, 0, 2).reshape(C, H, W))
    return np.stack(outs, axis=0).astype(np.float32)


_NC = {}


def get_module(repeat=1):
    if repeat not in _NC:
        _NC[repeat] = build_module(repeat)
    return _NC[repeat]


def kernel(**inputs) -> np.ndarray:
    from concourse.bass_utils import run_bass_kernel_spmd

    nc = get_module()
    in_maps = make_in_maps(**inputs)
    res = run_bass_kernel_spmd(nc, in_maps, core_ids=list(range(B)))
    return assemble_output(res.results)
